# revision 17
# baseline (speedup 1.0000x reference)
"""Multi-head distance (attention) layer on 8 TRN2 NeuronCores.

Sharding: data-parallel over batch. B=8 -> one batch element per core.
Each core computes full multi-head self-attention for its [L=1024, D=256]
slice with H=8 heads of dim 64. No collectives.

v2 design (vs the v1 single-exp-engine kernel):
  - Host folds the positional encoding into x (qk_in = x+pe) and feeds it
    TRANSPOSED ([d, l]) so the on-device PE transpose stage disappears;
    host also pre-scales Wq/bq by ALPHA = 2^10*log2e/8 so the score PSUM
    directly holds Y = 1024*log2(exp(s)) for both exp consumers.
  - Softmax exp is split across TWO engines: ScalarE runs Exp(scale*Y)
    with scale=ln2/1024; the Vector engine runs a custom 8-stage DVE op
    (hijacking the CODY_WAITE_CASCADE table row) that computes the fp16
    BIT PATTERN of 2^(Y/1024) via magic-constant floor + quadratic
    mantissa correction + int16-convert, written through an int16 bitcast
    of the fp16 e-tile. Generations are [128,1024] (= one S pair, 2 PSUM
    banks); 3 generation slots keep both engines fed.
  - All PSUM drains (K/Q/V projections) run on ScalarE activation-Copy
    (the Q bias rides the Copy's bias port); GpSimd has no PSUM access.
  - Output is NOT normalized on device: the raw [128, 4*65] pO quads
    (including the ones-column row sums) are copied by ScalarE to SBUF
    and DMA'd out as f32; the host divides and adds bv.
  - Inputs land via 6 large contiguous DMAs (per-partition rows >= 1KB).
"""

import numpy as np

import concourse.bass as bass
import concourse.mybir as mybir
import concourse.tile as tile
from concourse import bacc
from concourse.bass_utils import run_bass_kernel_spmd

B, L, D = 8, 1024, 256
H, HD = 8, 64
J = H * HD  # 512
TEMPERATURE = 10000.0

f32 = mybir.dt.float32
fp16 = mybir.dt.float16
i16 = mybir.dt.int16

_CACHE = {}
LAST_RESULT = None
TRACE = False

# ---------------------------------------------------------------------------
# custom DVE op: fp16 exp2 bit pattern (see module docstring)
# ---------------------------------------------------------------------------
LOG2E = 1.4426950408889634
P2, P0 = 0.34200922774724907, -0.08588165589152509  # c(f)~p2*(f-.5)^2+p0
ALPHA = float(np.float32(1024.0 * LOG2E / 8.0))
MAGIC = float(np.float32(3 * 2.0**32))              # fp32 ulp = 1024 here
B2C = float(np.float32(P2 / 1024.0))
B0C = float(np.float32(512.0 + 1024.0 * (P0 + 15.0)))  # rne output convert
LN2_1024 = float(np.float32(0.6931471805599453 / 1024.0))

_HIJACK = "CODY_WAITE_CASCADE"
_EXP2_OP = None


def _exp2_reference(in0, in1, s0, s1, imm2):
    g = np.float32
    Ym = (in0.astype(g) - g(imm2)).astype(g)
    t = (Ym + g(s0)).astype(g)
    K = (t - g(s0)).astype(g)
    F = (Ym - K).astype(g)
    Z = (Ym + in1.astype(g)).astype(g)
    u = (F * g(s1)).astype(g)
    w = (u * F).astype(g)
    return (Z + w).astype(g)


def _install_exp2():
    global _EXP2_OP
    if _EXP2_OP is not None:
        return _EXP2_OP
    import concourse.dve_ops as dv
    import concourse.bass_utils as bu
    from concourse.dve_spec import Spec, Src0, Src1, C0, C1, C2, Latch, lower
    from concourse.dve_uop import DveOpSpec
    from concourse.dve_ops import get_dve_sub_opcode

    Ym = Src0 - C2
    t = Ym + C0
    K = t - C0
    F = Ym - K
    Z = Ym + Latch(Src1)  # [P,1] operand: latch once at element 0
    u = F * C1
    w = u * F
    spec = Spec(body=Z + w, reference=_exp2_reference)
    shas = {}
    for ver in ("v3", "v4"):
        try:
            s = DveOpSpec(
                name=_HIJACK,
                opcode=get_dve_sub_opcode(_HIJACK),
                uops=lower(spec, ver=ver),
                rd1_en=True,
            )
            shas[ver] = s.sha(ver)
        except Exception:
            pass
    op = dv.DveOp(_HIJACK, spec, subdim=False, uops_sha=shas)
    for key in list(dv._COMPILE_CACHE):
        if key[0] == _HIJACK:
            del dv._COMPILE_CACHE[key]
    for key in list(bu._table_cache):
        if _HIJACK in key[1]:
            del bu._table_cache[key]
    dv.OPS[:] = [o for o in dv.OPS if o.name != _HIJACK] + [op]
    dv.CODY_WAITE_CASCADE = op
    _EXP2_OP = op
    return op


# ---------------------------------------------------------------------------
# schedule tables
# ---------------------------------------------------------------------------
# exp engine per (group, mc): 'A' = ScalarE Exp, 'V' = custom DVE exp2
PAT = [
    "VVAVVAVV",  # g0 (j0,l2=0) - ScalarE busy with head drains
    "AVAVAVAV",  # g1 (j0,l2=1)
    "AVAAVAVA",  # g2 (j1,l2=0)
    "AVAVAVAV",  # g3
    "AVAVAVAV",  # g4
    "AVAVAVAV",  # g5
    "AVAVAVAV",  # g6
    "AVAVAVAV",  # g7 - ends A,V so both engines finish together
]

# projection bodies launched after the s_pair of (group, mc)
PROJ = {
    (0, 0): [("k", 0, 1)],
    (0, 1): [("q", 0, 1)],
    (0, 2): [("v", 0)],
    (0, 3): [("v", 1)],
    (0, 4): [("v", 2)],
    (0, 5): [("v", 3), ("v", 4)],
    (0, 6): [("v", 5), ("v", 6)],
    (0, 7): [("v", 7)],
    (1, 0): [("k", 1, 0)],
    (1, 1): [("q", 1, 0)],
    (1, 2): [("k", 1, 1)],
    (1, 3): [("q", 1, 1)],
    (2, 0): [("k", 2, 0)],
    (2, 1): [("q", 2, 0)],
    (3, 0): [("k", 2, 1)],
    (3, 1): [("q", 2, 1)],
    (4, 0): [("k", 3, 0)],
    (4, 1): [("q", 3, 0)],
    (5, 0): [("k", 3, 1)],
    (5, 1): [("q", 3, 1)],
}

QUADS = {
    1: [(0, 0), (1, 0)],
    2: [(0, 1), (1, 1)],
    3: [(2, 0), (3, 0)],
    4: [(2, 1), (3, 1)],
    5: [(4, 0), (5, 0)],
    6: [(4, 1), (5, 1)],
    7: [(6, 0), (7, 0)],
}

N_WARM = 6


def _emit(tc, aps, bq_zero=False):
    nc = tc.nc
    Exp = mybir.ActivationFunctionType.Exp
    Copy = mybir.ActivationFunctionType.Copy
    op = _install_exp2()
    qkt, xt, wkq, wv, bqc, ou = (
        aps["qkt"], aps["xt"], aps["wkq"], aps["wv"], aps["bqc"], aps["ou"],
    )

    import contextlib
    ctx = contextlib.ExitStack()
    persist = ctx.enter_context(tc.tile_pool(name="persist", bufs=1))
    epool = ctx.enter_context(tc.tile_pool(name="epool", bufs=18))
    obpool = ctx.enter_context(tc.tile_pool(name="obpool", bufs=6))
    s_ps = ctx.enter_context(tc.tile_pool(name="sps", bufs=3, space="PSUM"))
    o_ps = ctx.enter_context(tc.tile_pool(name="ops", bufs=2, space="PSUM"))

    qkT = persist.tile([128, 2, 1024], fp16, name="qkT")
    xT = persist.tile([128, 2, 1024], fp16, name="xT")
    w_sb = persist.tile([128, 4, 512], fp16, name="w_sb")   # wk0 wk1 wq0 wq1
    wv_sb = persist.tile([128, 2, 512], fp16, name="wv_sb")
    bq_sb = persist.tile([128, 4], f32, name="bq_sb")
    b0_sb = persist.tile([128, 1], f32, name="b0_sb")
    warm_in = persist.tile([128, 512], fp16, name="warm_in")
    sc_in = persist.tile([128, 8], f32, name="sc_in")
    sc_out = persist.tile([128, 8], f32, name="sc_out")
    kT2 = [persist.tile([128, 1024], fp16, name=f"kT2{j}") for j in range(4)]
    qT = [persist.tile([128, 1024], fp16, name=f"qT{j}") for j in range(4)]
    v_sb = [persist.tile([128, 8, 65], fp16, name=f"v_sb{m}") for m in range(8)]

    # --- early memsets (gpsimd; no PSUM access but SBUF is fine) ---
    nc.gpsimd.memset(warm_in[:], 0.0)
    nc.gpsimd.memset(b0_sb[:], B0C)
    nc.gpsimd.memset(sc_in[:], 0.0)

    # --- input DMAs: few big transfers, both HWDGE rings used in parallel,
    # ordered by first use (K00 needs qkt-h1+wk; Q00 needs wq; K01 qkt-h2) ---
    # qk/x arrive in natural [l, d] layout; the DMA-transpose engine
    # (~3x the plain per-instruction DMA rate) writes them as [d-part, l]
    nc.sync.dma_start_transpose(out=qkT[:, 0, :], in_=qkt[:, 0:128])
    nc.scalar.dma_start(out=w_sb[:, 0:2, :], in_=wkq[:, 0:2, :])
    nc.gpsimd.dma_start(out=w_sb[:, 2:4, :], in_=wkq[:, 2:4, :])
    nc.sync.dma_start_transpose(out=qkT[:, 1, :], in_=qkt[:, 128:256])
    nc.sync.dma_start_transpose(out=xT[:, 0, :], in_=xt[:, 0:128])
    nc.sync.dma_start_transpose(out=xT[:, 1, :], in_=xt[:, 128:256])
    nc.sync.dma_start(out=bq_sb[:], in_=bqc[:, :])
    nc.gpsimd.dma_start(out=wv_sb[:], in_=wv[:, :, :])

    # --- ACT exp-table preload ---
    nc.scalar.activation(sc_out[:], sc_in[:], Exp)

    # --- v ones columns (row sums for the host-side normalize) ---
    for m in range(8):
        nc.gpsimd.memset(v_sb[m][:, :, 64:65], 1.0)

    # --- PE warm-up (keeps the HAM clock gate open through the DMA wait) ---
    warm_ps = o_ps.tile([128, 512], f32, tag="o", name="warm_ps")
    for _ in range(N_WARM):
        nc.tensor.matmul(
            warm_ps[:], lhsT=warm_in[:, 0:128], rhs=warm_in[:],
            start=True, stop=True,
        )

    # --- projection bodies ---
    def k_mm(j, l2, c):
        for c2 in range(2):
            nc.tensor.matmul(
                c,
                lhsT=w_sb[:, c2, j * 128:(j + 1) * 128],
                rhs=qkT[:, c2, l2 * 512:(l2 + 1) * 512],
                start=(c2 == 0), stop=(c2 == 1),
            )

    def q_mm(j, l2, c):
        for c2 in range(2):
            nc.tensor.matmul(
                c,
                lhsT=w_sb[:, 2 + c2, j * 128:(j + 1) * 128],
                rhs=qkT[:, c2, l2 * 512:(l2 + 1) * 512],
                start=(c2 == 0), stop=(c2 == 1),
            )

    def v_mm(m, c):
        for c2 in range(2):
            nc.tensor.matmul(
                c,
                lhsT=xT[:, c2, m * 128:(m + 1) * 128],
                rhs=wv_sb[:, c2, :],
                start=(c2 == 0), stop=(c2 == 1),
            )

    def proj(item):
        c = o_ps.tile([128, 512], f32, tag="o", name="pc")
        kind = item[0]
        if kind == "k":
            _, j, l2 = item
            k_mm(j, l2, c[:])
            nc.scalar.activation(kT2[j][:, l2 * 512:(l2 + 1) * 512], c[:], Copy)
        elif kind == "q":
            _, j, l2 = item
            q_mm(j, l2, c[:])
            if bq_zero:
                nc.vector.tensor_copy(qT[j][:, l2 * 512:(l2 + 1) * 512], c[:])
            else:
                nc.vector.tensor_scalar_add(
                    qT[j][:, l2 * 512:(l2 + 1) * 512], c[:], bq_sb[:, j:j + 1]
                )
        else:
            _, m = item
            v_mm(m, c[:])
            nc.scalar.activation(
                v_sb[m][:, :, 0:64],
                c.rearrange("p (h d) -> p h d", h=8),
                Copy,
            )

    # --- head: K(0,0) and Q(0,0) gate the first S pair ---
    proj(("k", 0, 0))
    proj(("q", 0, 0))

    # --- generations: one [128,1024] PSUM tile per S pair ---
    epos = {}

    def s_pair(j, mc, l2, eng):
        gen = s_ps.tile([128, 1024], f32, tag="s", name="sg")
        for half in range(2):
            rows = slice(64 * half, 64 * half + 64)
            nc.tensor.matmul(
                gen[:, half * 512:(half + 1) * 512],
                lhsT=kT2[j][rows, mc * 128:(mc + 1) * 128],
                rhs=qT[j][rows, l2 * 512:(l2 + 1) * 512],
                start=True, stop=True,
            )
        e = epool.tile([128, 1024], fp16, tag="e", name="e")
        if eng == "A":
            nc.scalar.activation(e[:], gen[:], Exp, scale=LN2_1024)
        else:
            nc.vector._custom_dve(
                op, out=e[:].bitcast(i16), in0=gen[:], in1=b0_sb[:, 0:1],
                s0=MAGIC, s1=B2C, imm2=512.0,
            )
        for half in range(2):
            epos[(2 * j + half, mc, l2)] = (e, 512 * half)

    # --- O quads ---
    oq = {}
    owork = []
    pending_fin = []
    pair_ob = {}

    def o_start(h, q):
        oq[(h, q)] = o_ps.tile([128, 260], f32, tag="o", name="pO")

    def enqueue_quad(h, q):
        o_start(h, q)
        owork.extend(("s", h, q, g, half) for g in range(4) for half in range(2))

    def emit_slice():
        if not owork:
            return
        _, h, q, g, half = owork.pop(0)
        pO = oq[(h, q)]
        for i in range(4):
            mc = 4 * half + i
            e, off = epos[(h, mc, q)]
            nc.tensor.matmul(
                pO[:, 65 * g:65 * g + 65],
                lhsT=e[:, off + g * 128:off + (g + 1) * 128],
                rhs=v_sb[mc][:, h, :],
                start=(half == 0 and i == 0),
                stop=(half == 1 and i == 3),
            )
        if g == 3 and half == 1:
            pending_fin.append((h, q))

    def o_chains(h, q, mc_hold=None):
        pO = oq[(h, q)]
        mcs = list(range(8))
        if mc_hold is not None:
            mcs = [m for m in mcs if m != mc_hold] + [mc_hold]
        for g in range(4):
            for i, mc in enumerate(mcs):
                e, off = epos[(h, mc, q)]
                nc.tensor.matmul(
                    pO[:, 65 * g:65 * g + 65],
                    lhsT=e[:, off + g * 128:off + (g + 1) * 128],
                    rhs=v_sb[mc][:, h, :],
                    start=(i == 0), stop=(i == 7),
                )

    def o_finish(h, q, split_dma=False):
        # pair pid = (h//2)*2 + q, slot = h%2; DMA once per completed pair
        pO = oq.pop((h, q))
        pid, slot = (h // 2) * 2 + q, h % 2
        if pid not in pair_ob:
            pair_ob[pid] = obpool.tile([128, 520], f32, tag="ob", name="ob")
        ob = pair_ob[pid]
        dst = ob[:, slot * 260:(slot + 1) * 260]
        if slot == 0:
            nc.scalar.activation(dst, pO[:], Copy)
        else:
            nc.vector.tensor_copy(dst, pO[:])
        if slot == 1:
            if split_dma:
                nc.sync.dma_start(out=ou[pid][:, 0:260], in_=ob[:, 0:260])
                nc.scalar.dma_start(out=ou[pid][:, 260:520], in_=ob[:, 260:520])
            else:
                eng = nc.gpsimd if pid in (1, 3) else nc.sync
                eng.dma_start(out=ou[pid], in_=ob[:])

    def flush_fin():
        while pending_fin:
            o_finish(*pending_fin.pop(0))

    # ---------------- main schedule ----------------
    groups = [(j, l2) for j in range(4) for l2 in range(2)]
    for gi, (j, l2) in enumerate(groups):
        for mc in range(8):
            if gi in QUADS:
                if mc == 0:
                    enqueue_quad(*QUADS[gi][0])
                elif mc == 4:
                    enqueue_quad(*QUADS[gi][1])
            flush_fin()
            s_pair(j, mc, l2, PAT[gi][mc])
            for item in PROJ.get((gi, mc), ()):
                proj(item)
            emit_slice()
            emit_slice()
        while owork:
            emit_slice()

    # ---------------- tail: heads 6/7, q=1 ----------------
    flush_fin()
    o_start(6, 1)
    o_start(7, 1)
    o_chains(6, 1, mc_hold=7)
    o_chains(7, 1, mc_hold=7)
    o_finish(6, 1)
    o_finish(7, 1, split_dma=True)

    ctx.close()


def _build(bq_zero):
    key = ("nc", bq_zero)
    if key in _CACHE:
        return _CACHE[key]
    _install_exp2()
    nc = bacc.Bacc("TRN2", target_bir_lowering=False, debug=False, num_devices=8)
    aps = {
        "qkt": nc.dram_tensor("qkt", [1024, 256], fp16, kind="ExternalInput").ap(),
        "xt": nc.dram_tensor("xt", [1024, 256], fp16, kind="ExternalInput").ap(),
        "wkq": nc.dram_tensor("wkq", [128, 4, 512], fp16, kind="ExternalInput").ap(),
        "wv": nc.dram_tensor("wv", [128, 2, 512], fp16, kind="ExternalInput").ap(),
        "bqc": nc.dram_tensor("bqc", [128, 4], f32, kind="ExternalInput").ap(),
        "ou": nc.dram_tensor("ou", [8, 128, 520], f32, kind="ExternalOutput").ap(),
    }
    with tile.TileContext(nc) as tc:
        _emit(tc, aps, bq_zero=bq_zero)
    nc.compile()
    _CACHE[key] = nc
    return nc


def _pe():
    embed = np.arange(L, dtype=np.float32)
    dim_t = np.arange(D, dtype=np.float32)
    dim_t = (np.float32(TEMPERATURE) ** (2.0 * np.floor(dim_t / 2.0) / np.float32(D))).astype(np.float32)
    pos = embed[:, None] / dim_t
    return np.stack([np.sin(pos[:, 0::2]), np.cos(pos[:, 1::2])], axis=2).reshape(L, D).astype(np.float32)


def _dt(a):  # [256, N] -> [128, 2, N] (d = t*128 + p)
    return np.ascontiguousarray(a.reshape(2, 128, -1).transpose(1, 0, 2))


def kernel(**inputs):
    global LAST_RESULT
    x = np.asarray(inputs["x"], dtype=np.float32)
    wq = np.asarray(inputs["Wq"], dtype=np.float32)
    wk = np.asarray(inputs["Wk"], dtype=np.float32)
    wv = np.asarray(inputs["Wv"], dtype=np.float32)
    bq = np.asarray(inputs["bq"], dtype=np.float32)
    bv = np.asarray(inputs["bv"], dtype=np.float32)

    nc = _build(not np.any(bq))
    pe = _pe()
    wkq = np.concatenate(
        [_dt(wk.astype(np.float16)), _dt((wq * np.float32(ALPHA)).astype(np.float16))],
        axis=1,
    )  # [128, 4, 512]
    wvp = _dt(wv.astype(np.float16))                       # [128, 2, 512]
    bqc = np.ascontiguousarray(
        (np.float32(ALPHA) * np.repeat(bq, HD)).reshape(4, 128).T
    ).astype(np.float32)                                    # [128, 4]
    base = {"wkq": wkq, "wv": wvp, "bqc": bqc}
    in_maps = []
    for b in range(B):
        in_maps.append({
            **base,
            "qkt": (x[b] + pe).astype(np.float16),         # [L, D] natural
            "xt": x[b].astype(np.float16),
        })
    res = run_bass_kernel_spmd(nc, in_maps, core_ids=list(range(B)), trace=TRACE)
    LAST_RESULT = res

    out = np.empty((B, L, J), np.float32)
    for b in range(B):
        # ou[pid= (h//2)*2+q, p, slot*260+260] with h = 2*(pid//2) + slot
        ouv = res.results[b]["ou"].reshape(4, 2, 128, 2, 4, 65)  # j2 q p s g c
        arr = ouv.transpose(1, 4, 2, 0, 3, 5).reshape(L, 8, 65)  # l (j2 s)=h c
        out[b] = (arr[:, :, 0:64] / arr[:, :, 64:65]).reshape(L, J)
    out += np.repeat(bv, HD)[None, None, :]
    return out


# revision 18
# speedup vs baseline: 1.1496x; 1.1496x over previous
"""Multi-head distance (attention) layer on 8 TRN2 NeuronCores.

Sharding: data-parallel over batch. B=8 -> one batch element per core.
Each core computes full multi-head self-attention for its [L=1024, D=256]
slice with H=8 heads of dim 64. No collectives.

v2 design (vs the v1 single-exp-engine kernel):
  - Host folds the positional encoding into x (qk_in = x+pe) and feeds it
    TRANSPOSED ([d, l]) so the on-device PE transpose stage disappears;
    host also pre-scales Wq/bq by ALPHA = 2^10*log2e/8 so the score PSUM
    directly holds Y = 1024*log2(exp(s)) for both exp consumers.
  - Softmax exp is split across TWO engines: ScalarE runs Exp(scale*Y)
    with scale=ln2/1024; the Vector engine runs a custom 8-stage DVE op
    (hijacking the CODY_WAITE_CASCADE table row) that computes the fp16
    BIT PATTERN of 2^(Y/1024) via magic-constant floor + quadratic
    mantissa correction + int16-convert, written through an int16 bitcast
    of the fp16 e-tile. Generations are [128,1024] (= one S pair, 2 PSUM
    banks); 3 generation slots keep both engines fed.
  - All PSUM drains (K/Q/V projections) run on ScalarE activation-Copy
    (the Q bias rides the Copy's bias port); GpSimd has no PSUM access.
  - Output is NOT normalized on device: the raw [128, 4*65] pO quads
    (including the ones-column row sums) are copied by ScalarE to SBUF
    and DMA'd out as f32; the host divides and adds bv.
  - Inputs land via 6 large contiguous DMAs (per-partition rows >= 1KB).
"""

import numpy as np

import concourse.bass as bass
import concourse.mybir as mybir
import concourse.tile as tile
from concourse import bacc
from concourse.bass_utils import run_bass_kernel_spmd

B, L, D = 8, 1024, 256
H, HD = 8, 64
J = H * HD  # 512
TEMPERATURE = 10000.0

f32 = mybir.dt.float32
fp16 = mybir.dt.float16
i16 = mybir.dt.int16

_CACHE = {}
LAST_RESULT = None
TRACE = False

# ---------------------------------------------------------------------------
# custom DVE op: fp16 exp2 bit pattern (see module docstring)
# ---------------------------------------------------------------------------
LOG2E = 1.4426950408889634
P2, P0 = 0.34200922774724907, -0.08588165589152509  # c(f)~p2*(f-.5)^2+p0
ALPHA = float(np.float32(1024.0 * LOG2E / 8.0))
MAGIC = float(np.float32(3 * 2.0**32))              # fp32 ulp = 1024 here
B2C = float(np.float32(P2 / 1024.0))
B0C = float(np.float32(512.0 + 1024.0 * (P0 + 15.0)))  # rne output convert
LN2_1024 = float(np.float32(0.6931471805599453 / 1024.0))

_HIJACK = "CODY_WAITE_CASCADE"
_EXP2_OP = None


def _exp2_reference(in0, in1, s0, s1, imm2):
    g = np.float32
    Ym = (in0.astype(g) - g(imm2)).astype(g)
    t = (Ym + g(s0)).astype(g)
    K = (t - g(s0)).astype(g)
    F = (Ym - K).astype(g)
    Z = (Ym + in1.astype(g)).astype(g)
    u = (F * g(s1)).astype(g)
    w = (u * F).astype(g)
    return (Z + w).astype(g)


def _install_exp2():
    global _EXP2_OP
    if _EXP2_OP is not None:
        return _EXP2_OP
    import concourse.dve_ops as dv
    import concourse.bass_utils as bu
    from concourse.dve_spec import Spec, Src0, Src1, C0, C1, C2, Latch, lower
    from concourse.dve_uop import DveOpSpec
    from concourse.dve_ops import get_dve_sub_opcode

    Ym = Src0 - C2
    t = Ym + C0
    K = t - C0
    F = Ym - K
    Z = Ym + Latch(Src1)  # [P,1] operand: latch once at element 0
    u = F * C1
    w = u * F
    spec = Spec(body=Z + w, reference=_exp2_reference)
    shas = {}
    for ver in ("v3", "v4"):
        try:
            s = DveOpSpec(
                name=_HIJACK,
                opcode=get_dve_sub_opcode(_HIJACK),
                uops=lower(spec, ver=ver),
                rd1_en=True,
            )
            shas[ver] = s.sha(ver)
        except Exception:
            pass
    op = dv.DveOp(_HIJACK, spec, subdim=False, uops_sha=shas)
    for key in list(dv._COMPILE_CACHE):
        if key[0] == _HIJACK:
            del dv._COMPILE_CACHE[key]
    for key in list(bu._table_cache):
        if _HIJACK in key[1]:
            del bu._table_cache[key]
    dv.OPS[:] = [o for o in dv.OPS if o.name != _HIJACK] + [op]
    dv.CODY_WAITE_CASCADE = op
    _EXP2_OP = op
    return op


# ---------------------------------------------------------------------------
# schedule tables
# ---------------------------------------------------------------------------
# exp engine per (group, mc): 'A' = ScalarE Exp, 'V' = custom DVE exp2
PAT = [
    "VVAVVAVV",  # g0 (j0,l2=0) - ScalarE busy with head drains
    "AVAVAVAV",  # g1 (j0,l2=1)
    "AVAAVAVA",  # g2 (j1,l2=0)
    "AVAVAVAV",  # g3
    "AVAVAVAV",  # g4
    "AVAVAVAV",  # g5
    "AVAVAVAV",  # g6
    "AVAVAVAV",  # g7 - ends A,V so both engines finish together
]

# projection bodies launched after the s_pair of (group, mc)
PROJ = {
    (0, 0): [("k", 0, 1)],
    (0, 1): [("q", 0, 1)],
    (0, 2): [("v", 0)],
    (0, 3): [("v", 1)],
    (0, 4): [("v", 2)],
    (0, 5): [("v", 3), ("v", 4)],
    (0, 6): [("v", 5), ("v", 6)],
    (0, 7): [("v", 7)],
    (1, 0): [("k", 1, 0)],
    (1, 1): [("q", 1, 0)],
    (1, 2): [("k", 1, 1)],
    (1, 3): [("q", 1, 1)],
    (2, 0): [("k", 2, 0)],
    (2, 1): [("q", 2, 0)],
    (3, 0): [("k", 2, 1)],
    (3, 1): [("q", 2, 1)],
    (4, 0): [("k", 3, 0)],
    (4, 1): [("q", 3, 0)],
    (5, 0): [("k", 3, 1)],
    (5, 1): [("q", 3, 1)],
}

QUADS = {
    1: [(0, 0), (1, 0)],
    2: [(0, 1), (1, 1)],
    3: [(2, 0), (3, 0)],
    4: [(2, 1), (3, 1)],
    5: [(4, 0), (5, 0)],
    6: [(4, 1), (5, 1)],
    7: [(6, 0), (7, 0)],
}

N_WARM = 5


def _emit(tc, aps, bq_zero=False):
    nc = tc.nc
    Exp = mybir.ActivationFunctionType.Exp
    Copy = mybir.ActivationFunctionType.Copy
    op = _install_exp2()
    qkt, xt, wkq, wv, bqc, ou = (
        aps["qkt"], aps["xt"], aps["wkq"], aps["wv"], aps["bqc"], aps["ou"],
    )

    import contextlib
    ctx = contextlib.ExitStack()
    persist = ctx.enter_context(tc.tile_pool(name="persist", bufs=1))
    epool = ctx.enter_context(tc.tile_pool(name="epool", bufs=18))
    obpool = ctx.enter_context(tc.tile_pool(name="obpool", bufs=6))
    s_ps = ctx.enter_context(tc.tile_pool(name="sps", bufs=3, space="PSUM"))
    o_ps = ctx.enter_context(tc.tile_pool(name="ops", bufs=2, space="PSUM"))

    qkT = persist.tile([128, 2, 1024], fp16, name="qkT")
    xT = persist.tile([128, 2, 1024], fp16, name="xT")
    w_sb = persist.tile([128, 4, 512], fp16, name="w_sb")   # wk0 wk1 wq0 wq1
    wv_sb = persist.tile([128, 2, 512], fp16, name="wv_sb")
    bq_sb = persist.tile([128, 4], f32, name="bq_sb")
    b0_sb = persist.tile([128, 1], f32, name="b0_sb")
    warm_in = persist.tile([128, 512], fp16, name="warm_in")
    sc_in = persist.tile([128, 8], f32, name="sc_in")
    sc_out = persist.tile([128, 8], f32, name="sc_out")
    kT2 = [persist.tile([128, 1024], fp16, name=f"kT2{j}") for j in range(4)]
    qT = [persist.tile([128, 1024], fp16, name=f"qT{j}") for j in range(4)]
    v_sb = [persist.tile([128, 8, 65], fp16, name=f"v_sb{m}") for m in range(8)]

    # --- early memsets (gpsimd; no PSUM access but SBUF is fine) ---
    nc.gpsimd.memset(warm_in[:], 0.0)
    nc.gpsimd.memset(b0_sb[:], B0C)
    nc.gpsimd.memset(sc_in[:], 0.0)

    # --- input DMAs: few big transfers, both HWDGE rings used in parallel,
    # ordered by first use (K00 needs qkt-h1+wk; Q00 needs wq; K01 qkt-h2) ---
    nc.sync.dma_start(out=qkT[:], in_=qkt[:, :, :])
    nc.scalar.dma_start(out=w_sb[:], in_=wkq[:, :, :])
    nc.gpsimd.dma_start(out=bq_sb[:], in_=bqc[:, :])
    nc.gpsimd.dma_start(out=xT[:], in_=xt[:, :, :])
    nc.gpsimd.dma_start(out=wv_sb[:], in_=wv[:, :, :])

    # --- ACT exp-table preload ---
    nc.scalar.activation(sc_out[:], sc_in[:], Exp)

    # --- v ones columns (row sums for the host-side normalize) ---
    for m in range(8):
        nc.gpsimd.memset(v_sb[m][:, :, 64:65], 1.0)

    # --- PE warm-up (keeps the HAM clock gate open through the DMA wait) ---
    warm_ps = o_ps.tile([128, 512], f32, tag="o", name="warm_ps")
    for _ in range(N_WARM):
        nc.tensor.matmul(
            warm_ps[:], lhsT=warm_in[:, 0:128], rhs=warm_in[:],
            start=True, stop=True,
        )

    # --- projection bodies ---
    def k_mm(j, l2, c):
        for c2 in range(2):
            nc.tensor.matmul(
                c,
                lhsT=w_sb[:, c2, j * 128:(j + 1) * 128],
                rhs=qkT[:, c2, l2 * 512:(l2 + 1) * 512],
                start=(c2 == 0), stop=(c2 == 1),
            )

    def q_mm(j, l2, c):
        for c2 in range(2):
            nc.tensor.matmul(
                c,
                lhsT=w_sb[:, 2 + c2, j * 128:(j + 1) * 128],
                rhs=qkT[:, c2, l2 * 512:(l2 + 1) * 512],
                start=(c2 == 0), stop=(c2 == 1),
            )

    def v_mm(m, c):
        for c2 in range(2):
            nc.tensor.matmul(
                c,
                lhsT=xT[:, c2, m * 128:(m + 1) * 128],
                rhs=wv_sb[:, c2, :],
                start=(c2 == 0), stop=(c2 == 1),
            )

    def proj(item):
        c = o_ps.tile([128, 512], f32, tag="o", name="pc")
        kind = item[0]
        if kind == "k":
            _, j, l2 = item
            k_mm(j, l2, c[:])
            nc.scalar.activation(kT2[j][:, l2 * 512:(l2 + 1) * 512], c[:], Copy)
        elif kind == "q":
            _, j, l2 = item
            q_mm(j, l2, c[:])
            if bq_zero:
                nc.vector.tensor_copy(qT[j][:, l2 * 512:(l2 + 1) * 512], c[:])
            else:
                nc.vector.tensor_scalar_add(
                    qT[j][:, l2 * 512:(l2 + 1) * 512], c[:], bq_sb[:, j:j + 1]
                )
        else:
            _, m = item
            v_mm(m, c[:])
            nc.scalar.activation(
                v_sb[m][:, :, 0:64],
                c.rearrange("p (h d) -> p h d", h=8),
                Copy,
            )

    # --- head: K(0,0) and Q(0,0) gate the first S pair ---
    proj(("k", 0, 0))
    proj(("q", 0, 0))

    # --- generations: one [128,1024] PSUM tile per S pair ---
    epos = {}

    def s_pair(j, mc, l2, eng):
        gen = s_ps.tile([128, 1024], f32, tag="s", name="sg")
        for half in range(2):
            rows = slice(64 * half, 64 * half + 64)
            nc.tensor.matmul(
                gen[:, half * 512:(half + 1) * 512],
                lhsT=kT2[j][rows, mc * 128:(mc + 1) * 128],
                rhs=qT[j][rows, l2 * 512:(l2 + 1) * 512],
                start=True, stop=True,
            )
        e = epool.tile([128, 1024], fp16, tag="e", name="e")
        if eng == "A":
            nc.scalar.activation(e[:], gen[:], Exp, scale=LN2_1024)
        else:
            nc.vector._custom_dve(
                op, out=e[:].bitcast(i16), in0=gen[:], in1=b0_sb[:, 0:1],
                s0=MAGIC, s1=B2C, imm2=512.0,
            )
        for half in range(2):
            epos[(2 * j + half, mc, l2)] = (e, 512 * half)

    # --- O quads ---
    oq = {}
    owork = []
    pending_fin = []
    pair_ob = {}

    def o_start(h, q):
        oq[(h, q)] = o_ps.tile([128, 260], f32, tag="o", name="pO")

    def enqueue_quad(h, q):
        o_start(h, q)
        owork.extend(("s", h, q, g, half) for g in range(4) for half in range(2))

    def emit_slice():
        if not owork:
            return
        _, h, q, g, half = owork.pop(0)
        pO = oq[(h, q)]
        for i in range(4):
            mc = 4 * half + i
            e, off = epos[(h, mc, q)]
            nc.tensor.matmul(
                pO[:, 65 * g:65 * g + 65],
                lhsT=e[:, off + g * 128:off + (g + 1) * 128],
                rhs=v_sb[mc][:, h, :],
                start=(half == 0 and i == 0),
                stop=(half == 1 and i == 3),
            )
        if g == 3 and half == 1:
            pending_fin.append((h, q))

    def o_chains(h, q, mc_hold=None):
        pO = oq[(h, q)]
        mcs = list(range(8))
        if mc_hold is not None:
            mcs = [m for m in mcs if m != mc_hold] + [mc_hold]
        for g in range(4):
            for i, mc in enumerate(mcs):
                e, off = epos[(h, mc, q)]
                nc.tensor.matmul(
                    pO[:, 65 * g:65 * g + 65],
                    lhsT=e[:, off + g * 128:off + (g + 1) * 128],
                    rhs=v_sb[mc][:, h, :],
                    start=(i == 0), stop=(i == 7),
                )

    def o_finish(h, q, split_dma=False):
        # pair pid = (h//2)*2 + q, slot = h%2; DMA once per completed pair
        pO = oq.pop((h, q))
        pid, slot = (h // 2) * 2 + q, h % 2
        if pid not in pair_ob:
            pair_ob[pid] = obpool.tile([128, 520], f32, tag="ob", name="ob")
        ob = pair_ob[pid]
        dst = ob[:, slot * 260:(slot + 1) * 260]
        if slot == 0:
            nc.scalar.activation(dst, pO[:], Copy)
        else:
            nc.vector.tensor_copy(dst, pO[:])
        if slot == 1:
            if split_dma:
                nc.sync.dma_start(out=ou[pid][:, 0:260], in_=ob[:, 0:260])
                nc.scalar.dma_start(out=ou[pid][:, 260:520], in_=ob[:, 260:520])
            else:
                eng = nc.gpsimd if pid in (1, 3) else nc.sync
                eng.dma_start(out=ou[pid], in_=ob[:])

    def flush_fin():
        while pending_fin:
            o_finish(*pending_fin.pop(0))

    # ---------------- main schedule ----------------
    groups = [(j, l2) for j in range(4) for l2 in range(2)]
    for gi, (j, l2) in enumerate(groups):
        for mc in range(8):
            if gi in QUADS:
                if mc == 0:
                    enqueue_quad(*QUADS[gi][0])
                elif mc == 4:
                    enqueue_quad(*QUADS[gi][1])
            flush_fin()
            s_pair(j, mc, l2, PAT[gi][mc])
            for item in PROJ.get((gi, mc), ()):
                proj(item)
            emit_slice()
            emit_slice()
        while owork:
            emit_slice()

    # ---------------- tail: heads 6/7, q=1 ----------------
    flush_fin()
    o_start(6, 1)
    o_start(7, 1)
    o_chains(6, 1, mc_hold=7)
    o_chains(7, 1, mc_hold=7)
    o_finish(6, 1)
    o_finish(7, 1, split_dma=True)

    ctx.close()


def _build(bq_zero):
    key = ("nc", bq_zero)
    if key in _CACHE:
        return _CACHE[key]
    _install_exp2()
    nc = bacc.Bacc("TRN2", target_bir_lowering=False, debug=False, num_devices=8)
    aps = {
        "qkt": nc.dram_tensor("qkt", [128, 2, 1024], fp16, kind="ExternalInput").ap(),
        "xt": nc.dram_tensor("xt", [128, 2, 1024], fp16, kind="ExternalInput").ap(),
        "wkq": nc.dram_tensor("wkq", [128, 4, 512], fp16, kind="ExternalInput").ap(),
        "wv": nc.dram_tensor("wv", [128, 2, 512], fp16, kind="ExternalInput").ap(),
        "bqc": nc.dram_tensor("bqc", [128, 4], f32, kind="ExternalInput").ap(),
        "ou": nc.dram_tensor("ou", [8, 128, 520], f32, kind="ExternalOutput").ap(),
    }
    with tile.TileContext(nc) as tc:
        _emit(tc, aps, bq_zero=bq_zero)
    nc.compile()
    _CACHE[key] = nc
    return nc


def _pe():
    embed = np.arange(L, dtype=np.float32)
    dim_t = np.arange(D, dtype=np.float32)
    dim_t = (np.float32(TEMPERATURE) ** (2.0 * np.floor(dim_t / 2.0) / np.float32(D))).astype(np.float32)
    pos = embed[:, None] / dim_t
    return np.stack([np.sin(pos[:, 0::2]), np.cos(pos[:, 1::2])], axis=2).reshape(L, D).astype(np.float32)


def _dt(a):  # [256, N] -> [128, 2, N] (d = t*128 + p)
    return np.ascontiguousarray(a.reshape(2, 128, -1).transpose(1, 0, 2))


def kernel(**inputs):
    global LAST_RESULT
    x = np.asarray(inputs["x"], dtype=np.float32)
    wq = np.asarray(inputs["Wq"], dtype=np.float32)
    wk = np.asarray(inputs["Wk"], dtype=np.float32)
    wv = np.asarray(inputs["Wv"], dtype=np.float32)
    bq = np.asarray(inputs["bq"], dtype=np.float32)
    bv = np.asarray(inputs["bv"], dtype=np.float32)

    nc = _build(not np.any(bq))
    pe = _pe()
    wkq = np.concatenate(
        [_dt(wk.astype(np.float16)), _dt((wq * np.float32(ALPHA)).astype(np.float16))],
        axis=1,
    )  # [128, 4, 512]
    wvp = _dt(wv.astype(np.float16))                       # [128, 2, 512]
    bqc = np.ascontiguousarray(
        (np.float32(ALPHA) * np.repeat(bq, HD)).reshape(4, 128).T
    ).astype(np.float32)                                    # [128, 4]
    base = {"wkq": wkq, "wv": wvp, "bqc": bqc}
    in_maps = []
    for b in range(B):
        qk = (x[b] + pe).astype(np.float16)                # [L, D]
        in_maps.append({
            **base,
            "qkt": _dt(np.ascontiguousarray(qk.T)),        # [128, 2, 1024]
            "xt": _dt(np.ascontiguousarray(x[b].T.astype(np.float16))),
        })
    res = run_bass_kernel_spmd(nc, in_maps, core_ids=list(range(B)), trace=TRACE)
    LAST_RESULT = res

    out = np.empty((B, L, J), np.float32)
    for b in range(B):
        # ou[pid= (h//2)*2+q, p, slot*260+260] with h = 2*(pid//2) + slot
        ouv = res.results[b]["ou"].reshape(4, 2, 128, 2, 4, 65)  # j2 q p s g c
        arr = ouv.transpose(1, 4, 2, 0, 3, 5).reshape(L, 8, 65)  # l (j2 s)=h c
        out[b] = (arr[:, :, 0:64] / arr[:, :, 64:65]).reshape(L, J)
    out += np.repeat(bv, HD)[None, None, :]
    return out
